# revision 1
# baseline (speedup 1.0000x reference)
"""Trainium2 Bass kernel for the e3nn-style equivariant 3D convolution.

Strategy:
  * The whole module (self-connection linear + radial-weight kernel
    generation + 5x5x5 conv, y = sc + 0.1*conv) collapses into ONE 3D
    convolution: the self-connection is a 1x1x1 conv, folded into the
    center tap of the 5x5x5 kernel. The tiny (5^3 x 64 x 64) kernel is
    built on host in numpy.
  * Data-parallel over the X axis across 8 NeuronCores: each core gets a
    12-plane input slab (8 output planes + 2-halo each side, host-padded)
    and produces an 8-plane output slab.
  * Per core, the conv is an implicit GEMM on the tensor engine with
    fp32r (fp22-mantissa, full-rate) matmuls:
      - SBUF x layout: per x-plane tile [128, 34, 68]: partition = channel
        + 64*(y parity), free = (y-pair row, z) with 2-voxel zero pads.
      - contraction K = 128 = 64 channels x 2 y-parities,
        stationary M = 128 = 64 out-channels x 2 output y-parities,
        moving N = 512 = 8 y-pair rows x 64 z.
      - All 125 taps of one output tile accumulate in a single PSUM bank
        via 75 matmuls (dx, dz, y-shift sigma in {-1,0,1}); dy is encoded
        in the 2x2 (parity x parity) block structure of each weight.
  * PSUM -> SBUF copy on the vector engine, DMA out, host concatenates.
"""

import math
import numpy as np

import concourse.bass as bass
import concourse.bacc as bacc
import concourse.mybir as mybir
from concourse import tile as tile_mod
from concourse.bass_utils import run_bass_kernel_spmd

# ---------------------------------------------------------------- constants
SIZE = 5
MUL = 16
DIM = 4 * MUL                  # 64 channels
INV_SQRT3 = 1.0 / math.sqrt(3.0)
ALPHA_0 = math.sqrt(1.0 / (2 * MUL))
ALPHA_1 = math.sqrt(3.0 / (2 * MUL))

B, C, X, Y, Z = 2, 64, 64, 64, 64
NCORE = 8
XO = X // NCORE                # output x-planes per core
XP = XO + 4                    # input x-planes per core (2-halo)
R = 68                         # padded z row width
YPR = 34                       # y-pair rows per parity (incl. 2 pad rows)
NBLK = 75                      # 5 dx * 5 dz * 3 sigma weight blocks
NSLOT = 7                      # rotating x-plane SBUF slots

F32 = mybir.dt.float32
F32R = mybir.dt.float32r


# ------------------------------------------------------- host-side weights
def _build_conv_weights(lin_w0, lin_w1, tp_weight):
    """Full folded conv kernel K_oi (64, 64, 5, 5, 5) fp64:
    y[b,o,x,y,z] = sum_{i,t} K_oi[o,i,tx,ty,tz] * x[b,i,x+tx-2,y+ty-2,z+tz-2]
    """
    r = np.linspace(-1.0, 1.0, SIZE, dtype=np.float64)
    gx, gy, gz = np.meshgrid(r, r, r, indexing='ij')
    lat = np.stack([gx, gy, gz], axis=-1)
    d = np.linalg.norm(lat, axis=-1)
    unit = np.where(d[..., None] > 0, lat / np.maximum(d[..., None], 1e-12), 0.0)
    sh0 = np.ones_like(d)
    sh1 = math.sqrt(3.0) * unit[..., [1, 2, 0]]

    sigma = 1.0 / (SIZE - 1)
    values = np.linspace(0.0, 1.0, SIZE)
    emb = np.exp(-(((d[..., None] - values) / sigma) ** 2)) / 1.12
    emb = emb @ tp_weight.astype(np.float64)
    emb = emb * (np.cos(math.pi * d) / SIZE ** 1.5)[..., None]
    mm = MUL * MUL
    g = (SIZE, SIZE, SIZE)
    wA = emb[..., 0*mm:1*mm].reshape(*g, MUL, MUL)
    wB = emb[..., 1*mm:2*mm].reshape(*g, MUL, MUL)
    wC = emb[..., 2*mm:3*mm].reshape(*g, MUL, MUL)
    wD = emb[..., 3*mm:4*mm].reshape(*g, MUL, MUL)

    eye3 = np.eye(3)
    k00 = ALPHA_0 * wA * sh0[..., None, None]
    k10 = (ALPHA_0 * INV_SQRT3) * np.einsum('...uw,...i->...uiw', wD, sh1)
    k01 = (ALPHA_1 * INV_SQRT3) * np.einsum('...uw,...k->...uwk', wB, sh1)
    k11 = (ALPHA_1 * INV_SQRT3) * np.einsum('...uw,ik->...uiwk',
                                            wC * sh0[..., None, None], eye3)
    top = np.concatenate([k00, k01.reshape(*g, MUL, 3*MUL)], axis=-1)
    bot = np.concatenate([k10.reshape(*g, 3*MUL, MUL),
                          k11.reshape(*g, 3*MUL, 3*MUL)], axis=-1)
    kernel = np.concatenate([top, bot], axis=-2)          # (5,5,5,in,out)
    K_oi = 0.1 * np.transpose(kernel, (4, 3, 0, 1, 2))    # (o,i,tx,ty,tz)

    # fold self connection (per-irrep linear) into the center tap
    inv = 1.0 / math.sqrt(MUL)
    L = np.zeros((DIM, DIM))
    L[:MUL, :MUL] = lin_w0.T.astype(np.float64) * inv
    iu = np.arange(MUL)
    for iv in range(3):
        L[(MUL + 3*iu[:, None] + iv), (MUL + 3*iu[None, :] + iv)] = \
            lin_w1.T.astype(np.float64) * inv
    K_oi[:, :, 2, 2, 2] += L
    return K_oi


def _pack_weight_blocks(K_oi):
    """(o,i,tx,ty,tz) -> (128, NBLK*128) fp32 stationary blocks.

    Block index bi = (dx*5 + dz)*3 + s, s = sigma+1.
    lhsT[c + 64*par, o + 64*q] = K_oi[o, c, dx, 2*sigma+par-q+2, dz].
    """
    W = np.zeros((128, NBLK * 128), dtype=np.float32)
    for dx in range(5):
        for dz in range(5):
            for s in range(3):
                sig = s - 1
                bi = (dx * 5 + dz) * 3 + s
                for par in range(2):
                    for q in range(2):
                        dy = 2 * sig + par - q + 2
                        if 0 <= dy < 5:
                            W[64*par:64*par+64,
                              bi*128 + 64*q: bi*128 + 64*q + 64] = \
                                K_oi[:, :, dx, dy, dz].T
    return W


# ------------------------------------------------------------- bass kernel
def _emit(tc, nc, x_ap, w_ap, y_ap, reps=1):
    # x_ap: [B, XP, 128, YPR, R] host-packed slot layout (pads included)
    yv = y_ap.rearrange("b o xo (yh par) z -> b o xo par yh z", par=2)

    with (
        tc.tile_pool(name="wpool", bufs=1) as wp,
        tc.tile_pool(name="xpool", bufs=1) as xp,
        tc.tile_pool(name="opool", bufs=4) as op,
        tc.tile_pool(name="pspool", bufs=6, space="PSUM") as pp,
        tc.tile_pool(name="scratchpool", bufs=1, space="PSUM") as scp,
    ):
        wt = wp.tile([128, NBLK, 128], F32R, tag="w")
        nc.sync.dma_start(out=wt[:, :, :], in_=w_ap.rearrange(
            "p (nb m) -> p nb m", m=128))

        slots = [xp.tile([128, YPR, R], F32R, tag=f"xs{i}", name=f"xs{i}")
                 for i in range(NSLOT)]

        # PE "observe" dummies: a 1-column matmul whose only dependency is a
        # just-issued DMA. The fp32r matmul encoding only supports ONE sync
        # wait, so every real matmul must reach walrus with <=1 wait; these
        # dummies advance PE's vector clock past fresh DMA ticks so real
        # group-start matmuls only ever wait on the PSUM-release (DVE) sem.
        scratch = scp.tile([128, 2], F32, tag="scratch", name="scratch")

        def observe(rhs_ap):
            # N=2: fp32r matmuls require an even innermost count on both the
            # moving operand and the PSUM destination.
            nc.tensor.matmul(scratch[:, :], wt[:, 0, :], rhs_ap,
                             start=True, stop=True)

        def load_plane(g, b, p):
            s = slots[g % NSLOT]
            nc.gpsimd.dma_start(out=s[:, :, :], in_=x_ap[b, p])
            return s

        first = True
        for rb in range(reps * B):
            rep, b = divmod(rb, B)
            pre = [load_plane(rb * XP + p, b, p) for p in range(4)]
            if first:
                observe(wt[:, 0, 0:2])
                first = False
            for s_t in pre:
                observe(s_t[:, 0, 0:2])
            for xo in range(XO):
                load_plane(rb * XP + xo + 4, b, xo + 4)
                planes = [slots[(rb * XP + xo + d) % NSLOT] for d in range(5)]
                for t in range(4):
                    yp0 = 8 * t + 1
                    pt = pp.tile([128, 8, 64], F32, tag="ps", name="pt")
                    k = 0
                    for dx in range(5):
                        s_t = planes[dx]
                        for dz in range(5):
                            for s in range(3):
                                sig = s - 1
                                bi = (dx * 5 + dz) * 3 + s
                                nc.tensor.matmul(
                                    pt[:, :, :],
                                    wt[:, bi, :],
                                    s_t[:, yp0+sig: yp0+sig+8, dz: dz+64],
                                    start=(k == 0),
                                    stop=(k == NBLK - 1),
                                )
                                k += 1
                    ot = op.tile([128, 8, 64], F32, tag="ot", name="ot")
                    nc.vector.tensor_copy(out=ot[:, :, :], in_=pt[:, :, :])
                    for q in range(2):
                        nc.gpsimd.dma_start(
                            out=yv[b, :, xo, q, 8*t: 8*t+8, :],
                            in_=ot[64*q: 64*q+64, :, :],
                        )


_CACHED_NC = None


def _build_nc():
    global _CACHED_NC
    if _CACHED_NC is not None:
        return _CACHED_NC
    nc = bacc.Bacc("TRN2", target_bir_lowering=False, debug=False,
                   num_swdge_queues=4)
    x_d = nc.dram_tensor("x", [B, XP, 128, YPR, R], F32R, kind="ExternalInput")
    w_d = nc.dram_tensor("w", [128, NBLK * 128], F32R, kind="ExternalInput")
    y_d = nc.dram_tensor("y", [B, C, XO, Y, Z], F32, kind="ExternalOutput")
    with tile_mod.TileContext(nc) as tc:
        _emit(tc, nc, x_d.ap(), w_d.ap(), y_d.ap())
    nc.compile()
    _CACHED_NC = nc
    return nc


# ---------------------------------------------------------------- entrypoint
TRACE = False          # set True (e.g. from test.py) to capture an NTFF profile
LAST_RESULT = None     # BassKernelResults of the most recent kernel() call


def _pack_shard(sh):
    """(B, C, XP, Y, Z) x-slab -> (B, XP, 128, YPR, R) padded slot layout.

    partition = channel + 64*(y parity); free = (y-pair row, z) with 2-voxel
    zero borders. Row yp holds y = 2*yp - 2 + par.
    """
    out = np.zeros((sh.shape[0], sh.shape[2], 128, YPR, R), dtype=np.float32)
    out[:, :, :64, 1:33, 2:66] = sh[:, :, :, 0::2, :].transpose(0, 2, 1, 3, 4)
    out[:, :, 64:, 1:33, 2:66] = sh[:, :, :, 1::2, :].transpose(0, 2, 1, 3, 4)
    return out


def kernel(x, lin_w0, lin_w1, tp_weight):
    x = np.ascontiguousarray(np.asarray(x, dtype=np.float32))
    K_oi = _build_conv_weights(np.asarray(lin_w0), np.asarray(lin_w1),
                               np.asarray(tp_weight))
    W = _pack_weight_blocks(K_oi)

    xpad = np.pad(x, ((0, 0), (0, 0), (2, 2), (0, 0), (0, 0)))
    in_maps = []
    for i in range(NCORE):
        in_maps.append({
            "x": _pack_shard(xpad[:, :, i*XO: i*XO + XP]),
            "w": W,
        })

    nc = _build_nc()
    kwargs = {}
    if TRACE:
        kwargs = {"trace": True, "trace_cores": list(range(NCORE))}
    res = run_bass_kernel_spmd(nc, in_maps, list(range(NCORE)), **kwargs)
    global LAST_RESULT
    LAST_RESULT = res
    out = np.concatenate([res.results[i]["y"] for i in range(NCORE)], axis=2)
    return out.astype(np.float32)



# revision 27
# speedup vs baseline: 1.9610x; 1.9610x over previous
"""Trainium2 Bass kernel for the e3nn-style equivariant 3D convolution.

Strategy (v2 — hybrid fp8-DoubleRow / fp32r):
  * The whole module (self-connection linear + radial-weight kernel
    generation + 5x5x5 conv, y = sc + 0.1*conv) collapses into ONE 3D
    convolution; the self-connection folds into the center tap. The tiny
    (5^3 x 64 x 64) kernel is built on host in numpy.
  * Data-parallel over X across 8 cores: each core consumes a 12-plane
    input slab (8 output planes + 2-halo) and emits 8 output planes.
  * Precision split: the center tap (dx,dy,dz)=(2,2,2) (+ its fp32r
    matmul block-mates) carries ~99.8% of the kernel's weight variance
    (it holds the folded self-connection). It is computed with ONE
    fp32r matmul per output tile. The remaining 123 taps have tiny
    weights and are computed in fp8e4m3 with DoubleRow perf mode
    (0.5 cycles/row — 2x the fp32r rate), quantization error lands at
    ~1.6e-3 max-rel, far under the 2e-2 gate.
  * fp8 DoubleRow matmul: contracts K=256 = (64ch x 2 y-parities) x
    (2 taps via the i-dim of the moving AP: a constant SBUF offset
    delta selects tap B = tap A + (ddx planes, ddz columns)); M=64
    out-channels, one output y-parity per matmul.
  * Moving windows are FULL padded rows so the (rows x width) window
    merges into a single contiguous AP dim (DoubleRow requires a 3-D
    moving AP [128, 2, N]): row stride 66 = [2 pad][64 data], the next
    row's pad provides the right pad; dz shifts are pure base-offset
    changes whose edge contamination lands only in discarded pad
    columns. Output tiles of 7 rows: N = 462 <= 512 PSUM bank.
  * Per output tile: 1 fp32r matmul (N cycles) + 76 DoubleRow matmuls
    (N/2 cycles each) -> ~0.54x the all-fp32r baseline's PE time.
"""

import math
import numpy as np
import ml_dtypes

import concourse.bass as bass
import concourse.bacc as bacc
import concourse.mybir as mybir
from concourse import tile as tile_mod
from concourse.bass_utils import run_bass_kernel_spmd

# ---------------------------------------------------------------- constants
SIZE = 5
MUL = 16
DIM = 4 * MUL                  # 64 channels
INV_SQRT3 = 1.0 / math.sqrt(3.0)
ALPHA_0 = math.sqrt(1.0 / (2 * MUL))
ALPHA_1 = math.sqrt(3.0 / (2 * MUL))

B, C, X, Y, Z = 2, 64, 64, 64, 64
NCORE = 8
XO = X // NCORE                # output x-planes per core
XP = XO + 4                    # input x-planes per core (2-halo)
R = 66                         # row stride: [2 pad][64 data]; right pad = next row's lead
NR = 34                        # y-pair rows per parity (1 pad + 32 + 1 pad)
PL = NR * R                    # plane stride (2244)
HDR = 2                        # leading pad so dz-2 offsets stay in-bounds
TAIL = 4
XFREE = HDR + XP * PL + TAIL   # per-batch fp8 slab bytes per partition
ROW_TILES = [(0, 7), (7, 7), (14, 7), (21, 7), (28, 4)]
SX = 32.0                      # fixed power-of-2 input scale for fp8

F32 = mybir.dt.float32
F32R = mybir.dt.float32r
F8 = mybir.dt.float8e4
FP8_NP = ml_dtypes.float8_e4m3
FP8_MAX = 224.0                # safe for both e4m3 (240) and e4m3fn (448)


# Three fp8 slab regions per batch in one SBUF tile (so every DoubleRow
# i-dim delta stays inside one allocation):
#   reg 0 "normal": partitions = (ch, y-parity)        — full py-pair taps
#   reg 1 "S":      partitions = (py0, py0 shifted +1) — pairs the odd
#                   py0-only taps (q=0's dy=4 row) with each other in K
#   reg 2 "V":      partitions = (py1, py1 shifted +1) — same for q=1's
#                   dy=0 row
NREG = 3


def _units(q):
    """Unit list for output parity q. A unit is one K-slot of a DoubleRow
    matmul: (reg, sig_eff, anchor(dx,dz), [(khalf, dx, dy, dz), ...]).
    sig_eff gives the moving-window row offset; each (khalf, ...) entry
    fills 64 stationary rows with that tap's weights."""
    units = []
    # full py-pair units in the normal region
    for sig in (-1, 0, 1):
        dys = [2 * sig + py - q + 2 for py in (0, 1)]
        if not all(0 <= dy < SIZE for dy in dys):
            continue                       # py-odd rows go to S/V regions
        for dx in range(SIZE):
            for dz in range(SIZE):
                if sig == 0 and dx == 2 and dz == 2:
                    continue
                units.append((0, sig, (dx, dz),
                              [(py, dx, dys[py], dz) for py in (0, 1)]))
    # the odd single-parity dy row, doubled up in its own region
    dy_odd = 4 if q == 0 else 0            # py0 for q=0, py1 for q=1
    reg = 1 if q == 0 else 2
    sig_odd = (q + dy_odd - 2 - (q + dy_odd) % 2) // 2
    for dx in range(SIZE):
        for dz in (0, 2):
            units.append((reg, sig_odd, (dx, dz),
                          [(0, dx, dy_odd, dz), (1, dx, dy_odd, dz + 1)]))
        units.append((reg, sig_odd, (dx, 4), [(0, dx, dy_odd, 4)]))
    return units


def _pairs(q):
    """Pair units into DoubleRow matmuls via the i-dim; only within the
    same (region, sigma) group so deltas keep the hardware-proven
    d(dx)*PL + d(dz) form. An odd leftover gets a zero i=1 half."""
    units = _units(q)
    groups = {}
    for u in units:
        groups.setdefault((u[0], u[1]), []).append(u)
    pairs = []
    for key in sorted(groups):
        g = sorted(groups[key], key=lambda u: u[2][0] * PL + u[2][1])
        for i in range(0, len(g), 2):
            pairs.append((g[i], g[i + 1] if i + 1 < len(g) else None))
    return pairs


PAIRS = {0: _pairs(0), 1: _pairs(1)}
NB = len(PAIRS[0]) + len(PAIRS[1])                                # 66


# ------------------------------------------------------- host-side weights
def _build_conv_weights(lin_w0, lin_w1, tp_weight):
    """Full folded conv kernel K_oi (64, 64, 5, 5, 5) fp64:
    y[b,o,x,y,z] = sum_{i,t} K_oi[o,i,tx,ty,tz] * x[b,i,x+tx-2,y+ty-2,z+tz-2]
    """
    r = np.linspace(-1.0, 1.0, SIZE, dtype=np.float64)
    gx, gy, gz = np.meshgrid(r, r, r, indexing='ij')
    lat = np.stack([gx, gy, gz], axis=-1)
    d = np.linalg.norm(lat, axis=-1)
    unit = np.where(d[..., None] > 0, lat / np.maximum(d[..., None], 1e-12), 0.0)
    sh0 = np.ones_like(d)
    sh1 = math.sqrt(3.0) * unit[..., [1, 2, 0]]

    sigma = 1.0 / (SIZE - 1)
    values = np.linspace(0.0, 1.0, SIZE)
    emb = np.exp(-(((d[..., None] - values) / sigma) ** 2)) / 1.12
    emb = emb @ tp_weight.astype(np.float64)
    emb = emb * (np.cos(math.pi * d) / SIZE ** 1.5)[..., None]
    mm = MUL * MUL
    g = (SIZE, SIZE, SIZE)
    wA = emb[..., 0*mm:1*mm].reshape(*g, MUL, MUL)
    wB = emb[..., 1*mm:2*mm].reshape(*g, MUL, MUL)
    wC = emb[..., 2*mm:3*mm].reshape(*g, MUL, MUL)
    wD = emb[..., 3*mm:4*mm].reshape(*g, MUL, MUL)

    eye3 = np.eye(3)
    k00 = ALPHA_0 * wA * sh0[..., None, None]
    k10 = (ALPHA_0 * INV_SQRT3) * np.einsum('...uw,...i->...uiw', wD, sh1)
    k01 = (ALPHA_1 * INV_SQRT3) * np.einsum('...uw,...k->...uwk', wB, sh1)
    k11 = (ALPHA_1 * INV_SQRT3) * np.einsum('...uw,ik->...uiwk',
                                            wC * sh0[..., None, None], eye3)
    top = np.concatenate([k00, k01.reshape(*g, MUL, 3*MUL)], axis=-1)
    bot = np.concatenate([k10.reshape(*g, 3*MUL, MUL),
                          k11.reshape(*g, 3*MUL, 3*MUL)], axis=-1)
    kernel = np.concatenate([top, bot], axis=-2)          # (5,5,5,in,out)
    K_oi = 0.1 * np.transpose(kernel, (4, 3, 0, 1, 2))    # (o,i,tx,ty,tz)

    # fold self connection (per-irrep linear) into the center tap
    inv = 1.0 / math.sqrt(MUL)
    L = np.zeros((DIM, DIM))
    L[:MUL, :MUL] = lin_w0.T.astype(np.float64) * inv
    iu = np.arange(MUL)
    for iv in range(3):
        L[(MUL + 3*iu[:, None] + iv), (MUL + 3*iu[None, :] + iv)] = \
            lin_w1.T.astype(np.float64) * inv
    K_oi[:, :, 2, 2, 2] += L
    return K_oi


def _pack_weights(K_oi):
    """-> (W8 [128, NB, 2, 64] fp8, Wc [128, 128] f32, out_scale f32).

    W8[c + 64*py, bi, i, o] = sW * K_oi[o, c, dx_i, 2*sig+py-q+2, dz_i]
    Wc[c + 64*py, o + 64*q] = sW*SX * K_oi[o, c, 2, 2+py-q, 2]
    """
    W64 = np.zeros((128, NB, 2, 64))
    bi = 0
    for q in (0, 1):
        for unitA, unitB in PAIRS[q]:
            for i, unit in enumerate((unitA, unitB)):
                if unit is None:
                    continue
                for kh, dx, dy, dz in unit[3]:
                    W64[64*kh:64*kh+64, bi, i, :] = K_oi[:, :, dx, dy, dz].T
            bi += 1
    assert bi == NB
    sW = FP8_MAX / max(np.abs(W64).max(), 1e-30)
    W8 = (W64 * sW).astype(FP8_NP)

    Wc = np.zeros((128, 128))
    for py in (0, 1):
        for q in (0, 1):
            Wc[64*py:64*py+64, 64*q:64*q+64] = K_oi[:, :, 2, 2 + py - q, 2].T
    Wc = (Wc * (sW * SX)).astype(np.float32)
    return W8, Wc, np.float32(1.0 / (sW * SX))


# ------------------------------------------------------------- bass kernel
def _emit(tc, nc, x8_ap, xc_ap, w8_ap, wc_ap, sc_ap, y_ap):
    yv = y_ap.rearrange("b o xo (yh par) z -> b o xo par yh z", par=2)

    with (
        tc.tile_pool(name="wpool", bufs=1) as wp,
        tc.tile_pool(name="xpool", bufs=1) as xp,
        tc.tile_pool(name="xcpool", bufs=2) as xcp,
        tc.tile_pool(name="opool", bufs=4) as op,
        tc.tile_pool(name="pspool", bufs=6, space="PSUM") as pp,
        tc.tile_pool(name="scratchpool", bufs=1, space="PSUM") as scp,
    ):
        wt8 = wp.tile([128, NB, 2, 64], F8, tag="w8")
        nc.sync.dma_start(out=wt8[:, :, :, :], in_=w8_ap.rearrange(
            "p (nb i m) -> p nb i m", i=2, m=64))
        wtc = wp.tile([128, 128], F32R, tag="wc")
        nc.sync.dma_start(out=wtc[:, :], in_=wc_ap)
        sct = wp.tile([128, 1], F32, tag="sc")
        nc.sync.dma_start(out=sct[:, :], in_=sc_ap)
        xt8 = xp.tile([128, NREG * B * XFREE], F8, tag="x8")
        for rb in range(NREG * B):
            nc.sync.dma_start(out=xt8[:, rb*XFREE:(rb+1)*XFREE],
                              in_=x8_ap[:, rb*XFREE:(rb+1)*XFREE])

        # PE "observe" dummies: 1-2 column matmuls whose only dependency is
        # a just-issued DMA. The matmul encoding supports a single sync
        # wait, so every real matmul must reach walrus with <=1 wait; these
        # advance PE's vector clock past fresh DMA ticks so real
        # group-start matmuls only ever wait on the PSUM-release sem.
        scratch = scp.tile([128, 2], F32, tag="scratch", name="scratch")

        def observe8(rhs):
            nc.tensor.matmul(scratch[:, :], wt8[:, 0, :, :].rearrange(
                "p i m -> p (i m)"), rhs, start=True, stop=True)

        def observeR(rhs):
            nc.tensor.matmul(scratch[:, :], wtc[:, :], rhs,
                             start=True, stop=True)

        observeR(wtc[:, 0:2])
        observe8(xt8[:, 0:2])

        part_stride = NREG * B * XFREE

        def dr_rhs(off, delta, n):
            rhs = xt8[:, 0:1].copy()
            rhs.ap = mybir.VecI64Pair([[part_stride, 128], [delta, 2], [1, n]])
            rhs.offset = rhs.offset + off
            return rhs

        for b in range(B):
            for xo in range(XO):
                xct = xcp.tile([128, PL], F32R, tag="xc", name="xct")
                nc.gpsimd.dma_start(out=xct[:, :], in_=xc_ap[b, xo])
                observeR(xct[:, 0:2])
                for r0, rt in ROW_TILES:
                    n = rt * R
                    k = 0
                    for q in (0, 1):
                        # DoubleRow dst must sit at partition base 0
                        # (walrus s3d3_mm_valid_dst_partition), so each
                        # output parity accumulates in its own PSUM tile.
                        pt = pp.tile([64, 512], F32, tag="ps", name="pt")
                        nc.tensor.matmul(
                            pt[:, 0:n], wtc[:, 64*q:64*q+64],
                            xct[:, (r0 + 1) * R: (r0 + 1) * R + n],
                            start=True, stop=False)
                        nq = len(PAIRS[q])
                        for kq, (unitA, unitB) in enumerate(PAIRS[q]):
                            regA, sigA, (dxA, dzA), _ = unitA
                            off = ((regA * B + b) * XFREE + HDR
                                   + (xo + dxA) * PL
                                   + (r0 + 1 + sigA) * R + dzA - 2)
                            if unitB is None:
                                delta = 0
                            else:
                                _, _, (dxB, dzB), _ = unitB
                                delta = (dxB - dxA) * PL + (dzB - dzA)
                            nc.tensor.matmul(
                                pt[:, 0:n],
                                wt8[:, k, :, :],
                                dr_rhs(off, delta, n),
                                start=False, stop=(kq == nq - 1),
                                perf_mode=mybir.MatmulPerfMode.DoubleRow)
                            k += 1
                        ot = op.tile([64, 7, R], F32, tag="ot", name="ot")
                        nc.scalar.activation(
                            out=ot[:, 0:rt, :], in_=pt[:, 0:n],
                            func=mybir.ActivationFunctionType.Copy,
                            scale=sct[0:64, 0:1])
                        nc.gpsimd.dma_start(
                            out=yv[b, :, xo, q, r0:r0+rt, :],
                            in_=ot[:, 0:rt, 2:66])


_CACHED_NC = None


def _build_nc():
    global _CACHED_NC
    if _CACHED_NC is not None:
        return _CACHED_NC
    nc = bacc.Bacc("TRN2", target_bir_lowering=False, debug=False,
                   num_swdge_queues=4)
    x8_d = nc.dram_tensor("x8", [128, NREG * B * XFREE], F8,
                          kind="ExternalInput")
    xc_d = nc.dram_tensor("xc", [B, XO, 128, PL], F32R, kind="ExternalInput")
    w8_d = nc.dram_tensor("w8", [128, NB * 2 * 64], F8, kind="ExternalInput")
    wc_d = nc.dram_tensor("wc", [128, 128], F32R, kind="ExternalInput")
    sc_d = nc.dram_tensor("sc", [128, 1], F32, kind="ExternalInput")
    y_d = nc.dram_tensor("y", [B, C, XO, Y, Z], F32, kind="ExternalOutput")
    with tile_mod.TileContext(nc) as tc:
        _emit(tc, nc, x8_d.ap(), xc_d.ap(), w8_d.ap(), wc_d.ap(),
              sc_d.ap(), y_d.ap())
    nc.compile()
    _CACHED_NC = nc
    return nc


# ---------------------------------------------------------------- entrypoint
TRACE = False          # set True (e.g. from test.py) to capture an NTFF profile
LAST_RESULT = None     # BassKernelResults of the most recent kernel() call


def _pack_rows(sh):
    """(B, C, P, Y, Z) slab -> (B, P, 128, NR, R) padded row layout, f32.

    partition = channel + 64*(y parity); free rows: row r holds
    y = 2*(r-1) + par at columns 2..65; rows 0/33 and columns 0/1 are zero
    (the right pad of a row is the next row's lead pad).
    """
    nb, _, p = sh.shape[:3]
    out = np.zeros((nb, p, 128, NR, R), dtype=np.float32)
    out[:, :, :64, 1:33, 2:66] = sh[:, :, :, 0::2, :].transpose(0, 2, 1, 3, 4)
    out[:, :, 64:, 1:33, 2:66] = sh[:, :, :, 1::2, :].transpose(0, 2, 1, 3, 4)
    return out


def kernel(x, lin_w0, lin_w1, tp_weight):
    x = np.ascontiguousarray(np.asarray(x, dtype=np.float32))
    K_oi = _build_conv_weights(np.asarray(lin_w0), np.asarray(lin_w1),
                               np.asarray(tp_weight))
    W8, Wc, out_scale = _pack_weights(K_oi)
    sc = np.full((128, 1), out_scale, dtype=np.float32)

    xpad = np.pad(x, ((0, 0), (0, 0), (2, 2), (0, 0), (0, 0)))
    in_maps = []
    for i in range(NCORE):
        slab = xpad[:, :, i*XO: i*XO + XP]                  # (B, C, 12, Y, Z)
        rows = _pack_rows(slab)                             # (B, 12, 128, NR, R)
        norm = np.zeros((128, B, XFREE), dtype=np.float32)
        norm[:, :, HDR:HDR + XP*PL] = np.clip(
            rows.transpose(2, 0, 1, 3, 4).reshape(128, B, XP*PL) * SX,
            -FP8_MAX, FP8_MAX)
        # S region: (py0, py0 shifted +1); V region: (py1, py1 shifted +1)
        x8 = np.zeros((128, NREG, B, XFREE), dtype=np.float32)
        x8[:, 0] = norm
        for reg, base in ((1, 0), (2, 64)):
            x8[0:64, reg] = norm[base:base+64]
            x8[64:128, reg, :, :-1] = norm[base:base+64, :, 1:]
        x8 = x8.reshape(128, NREG * B * XFREE).astype(FP8_NP)
        xc = rows[:, 2:2+XO].reshape(B, XO, 128, PL).copy()
        in_maps.append({
            "x8": x8,
            "xc": xc.astype(np.float32),
            "w8": W8.reshape(128, NB * 2 * 64),
            "wc": Wc,
            "sc": sc,
        })

    nc = _build_nc()
    kwargs = {}
    if TRACE:
        kwargs = {"trace": True, "trace_cores": list(range(NCORE))}
    res = run_bass_kernel_spmd(nc, in_maps, list(range(NCORE)), **kwargs)
    global LAST_RESULT
    LAST_RESULT = res
    out = np.concatenate([res.results[i]["y"] for i in range(NCORE)], axis=2)
    return out.astype(np.float32)
